# revision 3
# baseline (speedup 1.0000x reference)
"""GNN message passing on 8 TRN2 cores — single-stage sel-matmul design.

Per core (dst-sharded, SPMD):
  - nodes: global degree sort, dealt round-robin; NPC=12544 (98 tiles).
  - edges grouped by (8-tile group, src block of 32768 rows); per (tile,
    block) segment padded to x128 so every 128-edge gather chunk maps to
    ONE dst tile.
  - one dma_gather per (group, block) (4 queues rotated) fetches exact
    edge rows [128, C, 64] f32.
  - DVE (batched per call, stride-0 broadcast views):
      Gb  = G * val          (val = 1/deg[dst] per edge; 0 for pads) -> bf16
      sel = (IOTA == nid)    (nid = dst node-in-tile per edge)       -> bf16
  - PE: S_t [64 feats, 128 nodes] (PSUM) += Gb_chunk.T @ sel_chunk
    accumulated over all chunks of tile t => agg.T pre-scaled by 1/deg.
  - epilogue per tile: aT=copy(S_t); P=WnT@aT+WsT@hsT; relu; transpose;
    square+rowsum; rsqrt; scale; batched store (same as prior kernel).
"""
import numpy as np
from contextlib import ExitStack

N_NODES = 100000
N_EDGES = 1600000
D = 64
N_CORES = 8
NPC = 12544
NT = NPC // 128            # 98 tiles
GROUP = 4                  # tiles per gather group
N_GRP = (NT + GROUP - 1) // GROUP   # 13 groups (12x8 + 1x2)
BLK_W = 32768
N_BLK = 4
TBL_ROWS = N_BLK * BLK_W

_cache = {}


def _prep(h_neigh, h_self, src, dst, W_neigh, W_self):
    src = np.asarray(src, dtype=np.int64)
    dst = np.asarray(dst, dtype=np.int64)
    h_neigh = np.asarray(h_neigh, dtype=np.float32)
    h_self = np.asarray(h_self, dtype=np.float32)

    deg = np.bincount(dst, minlength=N_NODES)
    order = np.argsort(-deg, kind="stable")
    n_ext = NPC * N_CORES
    order_ext = np.concatenate(
        [order, np.full(n_ext - N_NODES, N_NODES, dtype=np.int64)])
    core_nodes = [order_ext[c::N_CORES] for c in range(N_CORES)]
    inv_deg = (1.0 / np.maximum(deg, 1)).astype(np.float32)

    rank_of_node = np.empty(N_NODES + 1, dtype=np.int64)
    rank_of_node[order_ext[:n_ext]] = np.arange(n_ext)
    e_rank = rank_of_node[dst]
    e_core = e_rank % N_CORES
    e_pos = e_rank // N_CORES              # position within core [0, NPC)
    e_tile = e_pos // 128
    e_p = e_pos % 128
    e_blk = np.minimum(src // BLK_W, N_BLK - 1)
    e_loc = (src - e_blk * BLK_W).astype(np.int64)

    # per (core, tile, block) counts -> common padded segment sizes
    cnt = np.zeros((N_CORES, NT, N_BLK), dtype=np.int64)
    np.add.at(cnt, (e_core, e_tile, e_blk), 1)
    seg = ((cnt.max(axis=0) + 127) // 128) * 128       # [NT, N_BLK]
    seg[:, 0] = np.maximum(seg[:, 0], 128)             # every tile has >=1 chunk
    # call sizes per (group, block)
    gstart = [g * GROUP for g in range(N_GRP)] + [NT]
    call_len = np.zeros((N_GRP, N_BLK), dtype=np.int64)
    for g in range(N_GRP):
        call_len[g] = seg[gstart[g]:gstart[g + 1]].sum(axis=0)
    # chunk -> (tile, first, last) map per (group, block)
    chunk_maps = []
    for g in range(N_GRP):
        for b in range(N_BLK):
            cm = []
            for t in range(gstart[g], gstart[g + 1]):
                nch = int(seg[t, b]) // 128
                for k in range(nch):
                    cm.append(t)
            chunk_maps.append(((g, b), cm))
    # first/last chunk of each tile across emission order (b ascending)
    first_chunk = {}
    last_chunk = {}
    for (g, b), cm in chunk_maps:
        for k, t in enumerate(cm):
            key = (g, b, k)
            if t not in first_chunk:
                first_chunk[t] = key
            last_chunk[t] = key

    sched = dict(seg=seg.tolist(), call_len=call_len.tolist(),
                 gstart=gstart, chunk_maps=chunk_maps,
                 first_chunk=first_chunk, last_chunk=last_chunk)


    # ---- per-core streams ----
    L_TOT = int(call_len.sum())            # total padded idxs per core
    C_TOT = L_TOT // 128

    # table: bf16, each row duplicated to fill a 256B descriptor
    import ml_dtypes as _mld
    tbl = np.zeros((TBL_ROWS, 2 * D), dtype=_mld.bfloat16)
    hb = h_neigh.astype(_mld.bfloat16)
    tbl[:N_NODES, 0:D] = hb
    tbl[:N_NODES, D:2 * D] = hb

    Wn_T = np.ascontiguousarray(W_neigh.astype(np.float32).T)
    Ws_T = np.ascontiguousarray(W_self.astype(np.float32).T)
    import ml_dtypes
    BF = ml_dtypes.bfloat16
    wT = np.concatenate([Wn_T, Ws_T], axis=1).astype(BF)         # [64, 128]
    CMAX = int(call_len.max()) // 128
    iota = np.ascontiguousarray(
        np.broadcast_to(np.arange(128, dtype=np.float32),
                        (128, CMAX, 128)).reshape(128, CMAX * 128)).astype(BF)
    ident = np.eye(128, dtype=np.float32)

    h_self_ext = np.vstack([h_self, np.zeros((1, D), np.float32)])

    # segment offsets within each call, per (tile, block)
    seg_off = np.zeros((NT, N_BLK), dtype=np.int64)
    for g in range(N_GRP):
        for b in range(N_BLK):
            off = 0
            for t in range(gstart[g], gstart[g + 1]):
                seg_off[t, b] = off
                off += seg[t, b]
    call_off = np.zeros((N_GRP, N_BLK), dtype=np.int64)  # idx offset of call
    off = 0
    for g in range(N_GRP):
        for b in range(N_BLK):
            call_off[g, b] = off
            off += call_len[g, b]

    in_maps = []
    for c in range(N_CORES):
        m = e_core == c
        et, eb, ep, el = e_tile[m], e_blk[m], e_p[m], e_loc[m]
        ed = dst[m]
        # sort core edges by (block-major call order): key = global slot
        g_of_t = et // GROUP
        base = call_off[g_of_t, eb] + seg_off[et, eb]
        # order within segment arbitrary; use stable sort by base
        so = np.argsort(base, kind="stable")
        et, eb, ep, el, ed, base = (a[so] for a in (et, eb, ep, el, ed, base))
        # slot within segment: running index per (tile, block)
        o = np.arange(et.size) - np.searchsorted(base, base)
        slot = base + o                                   # global idx slot

        idx_flat = np.zeros(L_TOT, dtype=np.int64)
        nid_flat = np.full(L_TOT, 255.0, dtype=np.float32)  # pads match no node
        idx_flat[:] = (np.arange(L_TOT, dtype=np.int64) * 2621) % 4096
        idx_flat[slot] = el
        nid_flat[slot] = ep.astype(np.float32)

        w = idx_flat.astype(np.int16).reshape(-1, 16).T   # [16, L/16]
        idx_arr = np.tile(w, (8, 1)).copy()
        # column-major per chunk: edge e = cc*128 + p -> [p, cc]
        nid_arr = np.ascontiguousarray(
            nid_flat.reshape(C_TOT, 128).T).astype(BF)

        hsT = np.ascontiguousarray(
            h_self_ext[core_nodes[c]].T).astype(BF)        # [64, NPC]
        ivd_ext = np.concatenate([inv_deg, [0.0]]).astype(np.float32)
        ivd_bf = np.ascontiguousarray(np.broadcast_to(
            ivd_ext[core_nodes[c]][None, :], (D, NPC))).astype(BF)

        in_maps.append(dict(tbl=tbl, idx=idx_arr, nid=nid_arr,
                            hsT=hsT, ivd=ivd_bf, wT=wT, iota=iota,
                            ident=ident))

    meta = dict(core_nodes=core_nodes)
    return in_maps, sched, meta


def _patch_queue_aware_sems():
    from concourse import tile_sem_assignment as tsa
    from concourse import mybir
    if getattr(tsa.TileClockTick, "_qaware_patched", False):
        return
    orig = tsa.TileClockTick._assign_tick

    def _assign_tick_qaware(self, inst):
        qn = getattr(inst, "queue_num", None)
        if qn is not None and getattr(inst, "engine", None) == mybir.EngineType.Pool:
            self.next_sw_dma_idx = int(qn) % self.swdge_sem_count
        return orig(self, inst)

    tsa.TileClockTick._assign_tick = _assign_tick_qaware
    tsa.TileClockTick._qaware_patched = True


def _build(sched):
    import os
    import concourse.bacc as bacc
    import concourse.tile as tile
    from concourse import mybir

    _patch_queue_aware_sems()

    F32 = mybir.dt.float32
    BF16 = mybir.dt.bfloat16
    I16 = mybir.dt.int16
    AF = mybir.ActivationFunctionType
    AL = mybir.AluOpType

    seg = np.array(sched["seg"])
    call_len = np.array(sched["call_len"])
    gstart = sched["gstart"]
    chunk_maps = dict(((g, b), cm) for (g, b), cm in sched["chunk_maps"])
    first_chunk = sched["first_chunk"]
    last_chunk = sched["last_chunk"]

    L_TOT = int(call_len.sum())
    C_TOT = L_TOT // 128
    CMAX = int(call_len.max()) // 128

    nc = bacc.Bacc("TRN2", target_bir_lowering=False, num_swdge_queues=4,
                   dynamic_dma_scratch_size=int(
                       os.environ.get("KERNEL_SCRATCH", "49152")))
    tbl = nc.declare_dram_parameter("tbl", [TBL_ROWS, 2 * D], BF16,
                                    isOutput=False)
    idx = nc.declare_dram_parameter("idx", [128, L_TOT // 16], I16,
                                    isOutput=False)
    nid = nc.declare_dram_parameter("nid", [128, C_TOT], BF16, isOutput=False)
    ivd = nc.declare_dram_parameter("ivd", [D, NPC], BF16, isOutput=False)
    hsT = nc.declare_dram_parameter("hsT", [D, NPC], BF16, isOutput=False)
    wT = nc.declare_dram_parameter("wT", [D, 2 * D], BF16, isOutput=False)
    iota = nc.declare_dram_parameter("iota", [128, CMAX * 128], BF16,
                                     isOutput=False)
    ident = nc.declare_dram_parameter("ident", [128, 128], F32, isOutput=False)
    out = nc.declare_dram_parameter("out", [128, NT * D], F32, isOutput=True)
    DEBUG = int(os.environ.get("KERNEL_DEBUG", "0"))
    outd = None
    if DEBUG:
        outd = nc.declare_dram_parameter("outd", [D, NT * 128], F32,
                                         isOutput=True)

    with tile.TileContext(nc) as tc, ExitStack() as ctx:
        singles = ctx.enter_context(tc.tile_pool(name="singles", bufs=1))
        gp = ctx.enter_context(tc.tile_pool(name="g", bufs=5))
        sp = ctx.enter_context(tc.tile_pool(name="sel", bufs=3))
        wk = ctx.enter_context(tc.tile_pool(name="wk", bufs=3))
        op = ctx.enter_context(tc.tile_pool(name="og", bufs=2))
        psS = ctx.enter_context(tc.tile_pool(name="psS", bufs=4, space="PSUM"))
        psZ = ctx.enter_context(tc.tile_pool(name="psZ", bufs=2, space="PSUM"))
        psT = ctx.enter_context(tc.tile_pool(name="psT", bufs=2, space="PSUM"))

        idx_sb = singles.tile([128, L_TOT // 16], I16)
        nc.sync.dma_start(out=idx_sb[:], in_=idx[:])
        nid_sb = singles.tile([128, C_TOT], BF16)
        nc.sync.dma_start(out=nid_sb[:], in_=nid[:])
        ivd_sb = singles.tile([D, NPC], BF16)
        nc.sync.dma_start(out=ivd_sb[:], in_=ivd[:])
        hsT_sb = singles.tile([D, NPC], BF16)
        nc.sync.dma_start(out=hsT_sb[:], in_=hsT[:])
        wT_sb = singles.tile([D, 2 * D], BF16)
        nc.sync.dma_start(out=wT_sb[:], in_=wT[:])
        iota_sb = singles.tile([128, CMAX * 128], BF16)
        nc.sync.dma_start(out=iota_sb[:], in_=iota[:])
        ident_sb = singles.tile([128, 128], F32)
        nc.sync.dma_start(out=ident_sb[:], in_=ident[:])
        eps = singles.tile([128, 1], F32)
        nc.vector.memset(eps[:], 1e-30)

        qn = [0]

        def next_q():
            q = qn[0] % 4
            qn[0] += 1
            return q

        S_tiles = {}

        def do_call(g, b, col0):
            L = int(call_len[g, b])
            C = L // 128
            cm = chunk_maps[(g, b)]
            gt = gp.tile([128, CMAX, 2 * D], BF16, tag="g")
            nc.gpsimd.dma_gather(
                out_ap=gt[:, 0:C, :],
                in_ap=tbl[b * BLK_W:(b + 1) * BLK_W, :],
                idxs_ap=idx_sb[:, col0 // 16:(col0 + L) // 16],
                num_idxs=L,
                num_idxs_reg=L,
                elem_size=2 * D,
                single_packet=False,
                queue_num=next_q(),
            )
            cc0 = col0 // 128
            gb = gt
            sel = sp.tile([128, CMAX, 128], BF16, tag="sel")
            nc.vector.tensor_tensor(
                sel[:, 0:C, :],
                iota_sb[:, 0:C * 128].rearrange("p (c n) -> p c n", n=128),
                nid_sb[:, cc0:cc0 + C].unsqueeze(2).to_broadcast([128, C, 128]),
                AL.is_equal)
            for k, t in enumerate(cm):
                if t not in S_tiles:
                    S_tiles[t] = psS.tile([D, 128], F32, tag="S",
                                          name=f"S_{t}")
                nc.tensor.matmul(
                    out=S_tiles[t][:],
                    lhsT=gb[:, k, 0:D],
                    rhs=sel[:, k, :],
                    start=(first_chunk[t] == (g, b, k)),
                    stop=(last_chunk[t] == (g, b, k)),
                )

        def do_group_compute(g):
            t0, t1 = gstart[g], gstart[g + 1]
            og = op.tile([128, t1 - t0, D], F32, tag="og", name=f"og_{g}")
            for t in range(t0, t1):
                S_t = S_tiles.pop(t)
                aT = wk.tile([D, 128], BF16, tag="aT")
                nc.scalar.activation(aT[:], S_t[:], AF.Copy)
                nc.vector.tensor_tensor(
                    aT[:], aT[:], ivd_sb[:, t * 128:(t + 1) * 128], AL.mult)
                if DEBUG:
                    aTf = wk.tile([D, 128], F32, tag="aTf")
                    nc.vector.tensor_copy(aTf[:], aT[:])
                    nc.sync.dma_start(
                        out=outd[:, t * 128:(t + 1) * 128], in_=aTf[:])
                p_z = psZ.tile([D, 128], F32, tag="pz")
                nc.tensor.matmul(out=p_z[:], lhsT=wT_sb[:, 0:D],
                                 rhs=aT[:], start=True, stop=False)
                nc.tensor.matmul(out=p_z[:], lhsT=wT_sb[:, D:2 * D],
                                 rhs=hsT_sb[:, t * 128:(t + 1) * 128],
                                 start=False, stop=True)
                zT = wk.tile([D, 128], F32, tag="zT")
                nc.scalar.activation(zT[:], p_z[:], AF.Relu)
                p_zn = psT.tile([128, D], F32, tag="pzn")
                nc.tensor.transpose(out=p_zn[:], in_=zT[:],
                                    identity=ident_sb[0:D, 0:D])
                sq = wk.tile([128, D], F32, tag="sq")
                s = wk.tile([128, 1], F32, tag="s")
                nc.scalar.activation(sq[:], p_zn[:], AF.Square,
                                     accum_out=s[:])
                nrm = wk.tile([128, 1], F32, tag="nrm")
                nc.scalar.activation(nrm[:], s[:], AF.Sqrt, bias=eps[:])
                r = wk.tile([128, 1], F32, tag="r")
                nc.vector.reciprocal(r[:], nrm[:])
                nc.scalar.mul(og[:, t - t0, :], p_zn[:], r[:])
            nc.sync.dma_start(
                out=out[:, t0 * D:t1 * D],
                in_=og[:].rearrange("p r d -> p (r d)"))

        # emission: calls pipelined; epilogue(g) interleaved after calls(g)
        col = 0
        call_cols = {}
        for g in range(N_GRP):
            for b in range(N_BLK):
                call_cols[(g, b)] = col
                col += int(call_len[g, b])

        for g in range(N_GRP):
            for b in range(N_BLK):
                do_call(g, b, call_cols[(g, b)])
            do_group_compute(g)

    nc.compile()
    return nc


def kernel(h_neigh, h_self, src, dst, W_neigh, W_self):
    import os
    from concourse.bass_utils import run_bass_kernel_spmd

    in_maps, sched, meta = _prep(h_neigh, h_self, src, dst, W_neigh, W_self)
    key = str(sched["call_len"])
    if key not in _cache:
        _cache[key] = _build(sched)
    nc = _cache[key]

    trace = bool(int(os.environ.get("KERNEL_TRACE", "0")))
    res = run_bass_kernel_spmd(nc, in_maps, core_ids=list(range(N_CORES)),
                               trace=trace)
    kernel.last_exec_time_ns = res.exec_time_ns
    kernel.last_result = res

    out = np.zeros((N_NODES, D), dtype=np.float32)
    for c in range(N_CORES):
        nodes = meta["core_nodes"][c]
        dev = res.results[c]["out"]
        dev = dev.reshape(128, NT, D).transpose(1, 0, 2).reshape(NPC, D)
        valid = nodes < N_NODES
        out[nodes[valid]] = dev[valid]
    return out


def last_exec_time_ns():
    return getattr(kernel, "last_exec_time_ns", None)


kernel.last_result = None
